# revision 25
# baseline (speedup 1.0000x reference)
"""GatedGraphConv (3-layer, GRU) Bass kernel for 8 Trainium2 NeuronCores.

Strategy (v2):
  - Shard nodes (dst segments) across 8 cores (12500 each).
  - Message gather in split-fp16: the h table is stored row-major as
    [hi_fp16(128) | lo_fp16(128)] 512B rows; dma_gather pulls whole rows.
    Gathers are spread round-robin over 4 SWDGE queues so all 8 GpSimd Q7
    cores generate descriptors (4 pairs in parallel).
  - Segment sums via TensorE: per 128-slot tile, 3 fp16 matmuls against
    one-hot scatter matrices S_hi/S_lo (edge weights folded in, ew split
    hi/lo to fp16 pairs). S matrices are precomputed on the host and
    streamed from DRAM (no on-device S build).
  - GRU in feature-major fp32; W_l folded into W_ih on the host.
  - h written back as fp16 hi|lo rows; AllGather in two row-chunks (after
    window 12 and window 24) so the first chunk overlaps GRU compute.
    Gather indices are remapped to the chunked-allgather row layout.
"""

import os
import sys
import numpy as np

for _p in ("/opt/trn_rl_repo",):
    if _p not in sys.path:
        sys.path.append(_p)

# ---------------------------------------------------------------------------
# constants (hardcoded problem shape)
# ---------------------------------------------------------------------------
N = 100000          # nodes
D = 128             # feature dim
L = 3               # layers
C = 8               # cores
NC_ = N // C        # nodes per core (12500)
NCP = 12800         # nodes per core, padded to NW*WIN
WIN = 512           # dst nodes per window
NW = NCP // WIN     # windows per core (25)
G = WIN // 128      # 128-wide subgroups per window (4)
SB = 4              # src superblocks (int16 index limit)
NCELL = G * SB      # cells per window (16)

NAGA = 6656         # rows per core in allgather chunk A (windows 0-12)
NAGB = 3584         # chunk B (windows 13-19)
NAGC = NC_ - NAGA - NAGB  # 2260, chunk C (windows 20-24)
ROWSA = C * NAGA    # 53248
ROWSB = C * NAGB    # 28672
ROWSC = C * NAGC    # 18080
# global (remapped) row boundaries of the 4 gather superblocks
SB_BOUNDS = [0, 26624, 53248, 53248 + ROWSB, 100000]


def _ceil_div(a, b):
    return -(-a // b)


def _pos_of(g):
    """Remap global node id -> row position in the chunked-allgather layout."""
    c, r = g // NC_, g % NC_
    return np.where(
        r < NAGA, c * NAGA + r,
        np.where(r < NAGA + NAGB,
                 ROWSA + c * NAGB + (r - NAGA),
                 ROWSA + ROWSB + c * NAGC + (r - NAGA - NAGB)))


# ---------------------------------------------------------------------------
# host-side planning
# ---------------------------------------------------------------------------
def _plan(edge_index, edge_attr):
    src = np.asarray(edge_index[0], dtype=np.int64)
    dst = np.asarray(edge_index[1], dtype=np.int64)
    ew = np.asarray(edge_attr, dtype=np.float32)

    core = dst // NC_
    dst_local = dst - core * NC_
    w = dst_local // WIN
    gq = (dst_local % WIN) // 128
    rel = (dst_local % 128).astype(np.int64)

    spos = _pos_of(src)
    sb = ((spos >= SB_BOUNDS[1]).astype(np.int64)
          + (spos >= SB_BOUNDS[2]) + (spos >= SB_BOUNDS[3]))
    src_rel = (spos - np.asarray(SB_BOUNDS)[sb]).astype(np.int32)

    n_cells = NW * NCELL
    cell = ((w * G + gq) * SB + sb).astype(np.int64)
    key = core * n_cells + cell

    order = np.argsort(key, kind="stable")
    key_s = key[order]
    src_s = src_rel[order]
    rel_s = rel[order]
    ew_s = ew[order]

    counts = np.bincount(key_s, minlength=C * n_cells).reshape(C, n_cells)
    T = int(_ceil_div(max(1, counts.max()), 128))
    spc = T * 128

    # position of each edge within its cell
    starts = np.zeros(C * n_cells, dtype=np.int64)
    cc = counts.reshape(-1)
    starts[1:] = np.cumsum(cc)[:-1]
    pos = np.arange(len(key_s)) - starts[key_s]

    # gather indices, -1 padded (trailing; ucode trims them)
    idx_all = np.full((C * n_cells, spc), -1, dtype=np.int16)
    idx_all[key_s, pos] = src_s.astype(np.int16)
    idx_all[cc == 0, 0] = 0  # empty cell: one dummy row-0 gather
    idx_all = idx_all.reshape(C, n_cells, spc)

    # idx wrapped layout: slot i -> [i % 16, i // 16], replicated to 128 parts
    iw = idx_all.reshape(C, NW, NCELL, spc // 16, 16)
    iw = np.ascontiguousarray(np.moveaxis(iw, -1, -2))     # [C,NW,cells,16,spc/16]
    iw = np.moveaxis(iw, 2, 3)                              # [C,NW,16,cells,spc/16]
    iw = iw.reshape(C, NW, 16, NCELL * (spc // 16))
    idx_rep = np.ascontiguousarray(
        np.tile(iw, (1, 1, 8, 1)))                          # [C,NW,128,cells*spc/16]

    # S matrices, fp16, streamed: per (cell, ti) a [128, 256] block = S_hi|S_lo
    ew_hi = ew_s.astype(np.float16)
    ew_lo = (ew_s - ew_hi.astype(np.float32)).astype(np.float16)
    COLS = n_cells * T * 2 * 128
    s_host = np.zeros((C, 128, COLS), dtype=np.float16)
    ti = pos // 128
    p = pos % 128
    col_hi = (key_s % n_cells * T + ti) * 2 * 128 + rel_s
    flat = p * COLS + col_hi
    sflat = s_host.reshape(C, -1)
    ckey = key_s // n_cells
    # per-core put (cores have disjoint edge sets)
    for c in range(C):
        m = ckey == c
        np.put(sflat[c], flat[m], ew_hi[m])
        np.put(sflat[c], flat[m] + 128, ew_lo[m])

    cnt = np.maximum(counts, 1).astype(np.int32).reshape(C, 1, n_cells)
    return T, idx_rep, s_host, cnt


def _split16(a):
    hi = a.astype(np.float16)
    lo = (a - hi.astype(np.float32)).astype(np.float16)
    return hi, lo


# ---------------------------------------------------------------------------
# device program
# ---------------------------------------------------------------------------
def _build_program(T):
    from contextlib import ExitStack
    import concourse.bass as bass
    import concourse.tile as tile
    from concourse import bacc, mybir

    f32 = mybir.dt.float32
    f16 = mybir.dt.float16
    i16 = mybir.dt.int16
    add = mybir.AluOpType.add
    spc = T * 128
    COLW = NCELL * T * 2 * 128   # S columns per window

    nc = bacc.Bacc("TRN2", target_bir_lowering=False, debug=False,
                   num_devices=C, num_swdge_queues=4,
                   dynamic_dma_scratch_size=16384)

    x_ownT = nc.dram_tensor("x_ownT", [D, NCP], f32, kind="ExternalInput").ap()
    x_split = nc.dram_tensor("x_split", [N, 256], f16, kind="ExternalInput").ap()
    s_dram = nc.dram_tensor("s_dram", [128, NW * COLW], f16, kind="ExternalInput").ap()
    idx_dram = nc.dram_tensor("idx_dram", [NW, 128, NCELL * (spc // 16)], i16,
                              kind="ExternalInput").ap()
    wie_dram = nc.dram_tensor("wie_dram", [128, L * 3 * 128], f32, kind="ExternalInput").ap()
    whh_dram = nc.dram_tensor("whh_dram", [128, 3 * 128], f32, kind="ExternalInput").ap()
    bias_dram = nc.dram_tensor("bias_dram", [128, 4], f32, kind="ExternalInput").ap()
    ident_dram = nc.dram_tensor("ident_dram", [128, 128], f32, kind="ExternalInput").ap()
    i32 = mybir.dt.int32
    cnt_dram = nc.dram_tensor("cnt_dram", [1, NW * NCELL], i32, kind="ExternalInput").ap()

    out = nc.dram_tensor("out", [NC_, D], f32, kind="ExternalOutput").ap()

    with tile.TileContext(nc) as tc, ExitStack() as ctx:
        const = ctx.enter_context(tc.tile_pool(name="const", bufs=1))
        dram = ctx.enter_context(tc.tile_pool(name="dram", bufs=1, space="DRAM"))
        idxp = ctx.enter_context(tc.tile_pool(name="idxp", bufs=2))
        spool = ctx.enter_context(tc.tile_pool(name="spool", bufs=2))
        aggps = ctx.enter_context(tc.tile_pool(name="aggps", bufs=2, space="PSUM"))
        grups = ctx.enter_context(tc.tile_pool(name="grups", bufs=1, space="PSUM"))
        aggsb = ctx.enter_context(tc.tile_pool(name="aggsb", bufs=2))
        tmpp = ctx.enter_context(tc.tile_pool(name="tmpp", bufs=1))
        rowp = ctx.enter_context(tc.tile_pool(name="rowp", bufs=1))

        h_sb = const.tile([D, NCP], f32)
        ident_sb = const.tile([128, 128], f32)
        wie_sb = const.tile([128, L * 3 * 128], f32)
        whh_sb = const.tile([128, 3 * 128], f32)
        bias_sb = const.tile([128, 4], f32)

        nc.sync.dma_start(h_sb[:], x_ownT[:])
        nc.sync.dma_start(ident_sb[:], ident_dram[:])
        nc.sync.dma_start(wie_sb[:], wie_dram[:])
        nc.sync.dma_start(whh_sb[:], whh_dram[:])
        nc.sync.dma_start(bias_sb[:], bias_dram[:])
        cnt_sb = const.tile([1, NW * NCELL], i32)
        nc.sync.dma_start(cnt_sb[:], cnt_dram[:])

        NMSG = 11
        msg_bufs = [const.tile([128, T * 256], f16, name=f"msgbuf{i}")
                    for i in range(NMSG)]
        NREG = 8
        cnt_regs = [nc.gpsimd.alloc_register(f"cntreg{i}") for i in range(NREG)]
        for mb in msg_bufs:
            nc.vector.memset(mb[:], 0.0)

        bounceA = [dram.tile([NAGA, 256], f16, name=f"bA{l}") for l in range(2)]
        bounceB = [dram.tile([NAGB, 256], f16, name=f"bB{l}") for l in range(2)]
        bounceC = [dram.tile([NAGC, 256], f16, name=f"bC{l}") for l in range(2)]
        h_fullA = [dram.tile([ROWSA, 256], f16, name=f"hfA{l}", addr_space="Shared")
                   for l in range(2)]
        h_fullB = [dram.tile([ROWSB, 256], f16, name=f"hfB{l}", addr_space="Shared")
                   for l in range(2)]
        h_fullC = [dram.tile([ROWSC, 256], f16, name=f"hfC{l}", addr_space="Shared")
                   for l in range(2)]

        dbg_layers = int(os.environ.get("KDBG_LAYERS", str(L)))
        dbg_ag = os.environ.get("KDBG_AG", "1") == "1"

        def tables(l):
            if l == 0 or not dbg_ag:
                return [x_split[SB_BOUNDS[i]:SB_BOUNDS[i + 1], :] for i in range(4)]
            a, b, cc = h_fullA[l - 1], h_fullB[l - 1], h_fullC[l - 1]
            return [a[0:26624, :], a[26624:ROWSA, :], b[:, :], cc[:, :]]

        msg_i = 0
        q_rr = 0

        def do_agg(l, w, tab):
            nonlocal msg_i, q_rr
            s_ws = []
            for sq in range(4):
                swq = spool.tile([128, COLW // 4], f16, tag=f"sw{sq}")
                nc.sync.dma_start(
                    swq[:],
                    s_dram[:, w * COLW + sq * (COLW // 4):w * COLW + (sq + 1) * (COLW // 4)])
                s_ws.append(swq)
            idx_w = idxp.tile([128, NCELL * (spc // 16)], i16, tag="idx")
            nc.sync.dma_start(idx_w[:], idx_dram[w])

            pa = aggps.tile([128, 2 * WIN], f32, tag="agg")
            for gq in range(G):
                for sbi in range(SB):
                    ci = gq * SB + sbi
                    msg = msg_bufs[msg_i % NMSG]
                    msg_i += 1
                    creg = cnt_regs[msg_i % NREG]
                    nc.gpsimd.reg_load(creg, cnt_sb[0:1, w * NCELL + ci:w * NCELL + ci + 1])
                    nc.gpsimd.dma_gather(
                        msg.rearrange("p (t f) -> p t f", f=256),
                        tab[sbi],
                        idx_w[:, ci * (spc // 16):(ci + 1) * (spc // 16)],
                        spc, creg, 256,
                        queue_num=q_rr % 4,
                    )
                    q_rr += 1
                    po = pa[:, gq * 256:(gq + 1) * 256]
                    s_src = s_ws[ci // 4]
                    cio = ci % 4
                    for ti in range(T):
                        base = (cio * T + ti) * 2 * 128
                        mhi = msg[:, ti * 256:ti * 256 + 128]
                        mlo = msg[:, ti * 256 + 128:ti * 256 + 256]
                        spair = s_src[:, base:base + 256]
                        shi = s_src[:, base:base + 128]
                        first = (sbi == 0 and ti == 0)
                        last = (sbi == SB - 1 and ti == T - 1)
                        nc.tensor.matmul(po, lhsT=mhi, rhs=spair,
                                         start=first, stop=False,
                                         skip_group_check=True)
                        nc.tensor.matmul(po[:, 0:128], lhsT=mlo, rhs=shi,
                                         start=False, stop=last,
                                         skip_group_check=True)

            agg_sb = aggsb.tile([128, WIN], f32, tag="aggsb")
            for g2 in range(G):
                nc.vector.tensor_copy(
                    agg_sb[:, g2 * 128:(g2 + 1) * 128],
                    pa[:, g2 * 256:g2 * 256 + 128])
            for g2 in range(G):
                nc.vector.tensor_add(
                    agg_sb[:, g2 * 128:(g2 + 1) * 128],
                    agg_sb[:, g2 * 128:(g2 + 1) * 128],
                    pa[:, g2 * 256 + 128:(g2 + 1) * 256])
            return agg_sb

        def do_gru(l, w, agg_sb):
            cs = slice(w * WIN, (w + 1) * WIN)
            p_r = grups.tile([128, WIN], f32, tag="p_r")
            p_z = grups.tile([128, WIN], f32, tag="p_z")
            p_in = grups.tile([128, WIN], f32, tag="p_in")
            p_hn = grups.tile([128, WIN], f32, tag="p_hn")

            def wie(k):
                o = (l * 3 + k) * 128
                return wie_sb[:, o:o + 128]

            def whh(k):
                return whh_sb[:, k * 128:(k + 1) * 128]

            nc.tensor.matmul(p_r[:], lhsT=wie(0), rhs=agg_sb[:], start=True, stop=False)
            nc.tensor.matmul(p_r[:], lhsT=whh(0), rhs=h_sb[:, cs], start=False, stop=True)
            nc.tensor.matmul(p_z[:], lhsT=wie(1), rhs=agg_sb[:], start=True, stop=False)
            nc.tensor.matmul(p_z[:], lhsT=whh(1), rhs=h_sb[:, cs], start=False, stop=True)
            nc.tensor.matmul(p_in[:], lhsT=wie(2), rhs=agg_sb[:], start=True, stop=True)
            nc.tensor.matmul(p_hn[:], lhsT=whh(2), rhs=h_sb[:, cs], start=True, stop=True)

            r = tmpp.tile([128, WIN], f32, tag="tC")
            nc.scalar.activation(r[:], p_r[:], mybir.ActivationFunctionType.Sigmoid,
                                 bias=bias_sb[:, 0:1])
            z = tmpp.tile([128, WIN], f32, tag="tD")
            nc.scalar.activation(z[:], p_z[:], mybir.ActivationFunctionType.Sigmoid,
                                 bias=bias_sb[:, 1:2])
            hnb = tmpp.tile([128, WIN], f32, tag="tA")
            nc.vector.tensor_scalar(hnb[:], p_hn[:], bias_sb[:, 3:4], None, op0=add)
            rt = tmpp.tile([128, WIN], f32, tag="tB")
            nc.vector.tensor_mul(rt[:], r[:], hnb[:])
            s_ = tmpp.tile([128, WIN], f32, tag="tA")
            nc.vector.tensor_add(s_[:], p_in[:], rt[:])
            n_ = tmpp.tile([128, WIN], f32, tag="tC")
            nc.scalar.activation(n_[:], s_[:], mybir.ActivationFunctionType.Tanh,
                                 bias=bias_sb[:, 2:3])
            d_ = tmpp.tile([128, WIN], f32, tag="tA")
            nc.vector.tensor_sub(d_[:], h_sb[:, cs], n_[:])
            zd = tmpp.tile([128, WIN], f32, tag="tB")
            nc.vector.tensor_mul(zd[:], z[:], d_[:])
            nc.vector.tensor_add(h_sb[:, cs], n_[:], zd[:])

            p_t = grups.tile([128, WIN], f32, tag="p_r")
            for q in range(G):
                nc.tensor.transpose(
                    p_t[:, q * 128:(q + 1) * 128],
                    h_sb[:, w * WIN + q * 128: w * WIN + (q + 1) * 128],
                    ident_sb[:])

            if l < 2:
                hi16 = rowp.tile([128, WIN], f16, tag="hi16")
                nc.vector.tensor_copy(hi16[:], p_t[:])
                hi32 = rowp.tile([128, WIN], f32, tag="hr")
                nc.vector.tensor_copy(hi32[:], hi16[:])
                lo16 = rowp.tile([128, WIN], f16, tag="lo16")
                nc.vector.tensor_sub(lo16[:], p_t[:], hi32[:])

                if w <= 12:
                    dstt, r0, rmax = bounceA[l], w * WIN, NAGA
                elif w <= 19:
                    dstt, r0, rmax = bounceB[l], w * WIN - NAGA, NAGB
                else:
                    dstt, r0, rmax = bounceC[l], w * WIN - NAGA - NAGB, NAGC
                for half, ht in ((0, hi16), (1, lo16)):
                    h3 = ht.rearrange("p (q f) -> p q f", f=D)
                    fs = slice(half * 128, half * 128 + 128)
                    if r0 + WIN <= rmax:
                        dv = dstt[r0:r0 + WIN, fs].rearrange("(q p) f -> p q f", p=128)
                        nc.sync.dma_start(dv, h3)
                    else:
                        rem = rmax - r0
                        nq = rem // 128
                        if nq > 0:
                            dv = dstt[r0:r0 + nq * 128, fs].rearrange(
                                "(q p) f -> p q f", p=128)
                            nc.sync.dma_start(dv, h3[:, 0:nq, :])
                        rtail = rem - nq * 128
                        if rtail > 0:
                            dv = dstt[r0 + nq * 128:r0 + rem, fs].rearrange(
                                "(q p) f -> p q f", q=1)
                            nc.sync.dma_start(dv, h3[0:rtail, nq:nq + 1, :])

                if w == 12 and dbg_ag:
                    nc.gpsimd.collective_compute(
                        "AllGather", mybir.AluOpType.bypass,
                        replica_groups=[list(range(C))],
                        ins=[bounceA[l].opt()], outs=[h_fullA[l].opt()])
                if w == 19 and dbg_ag:
                    nc.gpsimd.collective_compute(
                        "AllGather", mybir.AluOpType.bypass,
                        replica_groups=[list(range(C))],
                        ins=[bounceB[l].opt()], outs=[h_fullB[l].opt()])
                if w == NW - 1 and dbg_ag:
                    nc.gpsimd.collective_compute(
                        "AllGather", mybir.AluOpType.bypass,
                        replica_groups=[list(range(C))],
                        ins=[bounceC[l].opt()], outs=[h_fullC[l].opt()])
            else:
                hr = rowp.tile([128, WIN], f32, tag="hr")
                nc.vector.tensor_copy(hr[:], p_t[:])
                hr3 = hr.rearrange("p (q f) -> p q f", f=D)
                r0 = w * WIN
                if r0 + WIN <= NC_:
                    dv = out[r0:r0 + WIN, :].rearrange("(q p) f -> p q f", p=128)
                    nc.sync.dma_start(dv, hr3)
                else:
                    rem = NC_ - r0
                    nq = rem // 128
                    if nq > 0:
                        dv = out[r0:r0 + nq * 128, :].rearrange(
                            "(q p) f -> p q f", p=128)
                        nc.sync.dma_start(dv, hr3[:, 0:nq, :])
                    rtail = rem - nq * 128
                    if rtail > 0:
                        dv = out[r0 + nq * 128:r0 + rem, :].rearrange(
                            "(q p) f -> p q f", q=1)
                        nc.sync.dma_start(dv, hr3[0:rtail, nq:nq + 1, :])

        for l in range(dbg_layers):
            tab = tables(l)
            prev = None
            for w in range(int(os.environ.get("KDBG_NW", str(NW)))):
                if prev is not None:
                    do_gru(l, prev[0], prev[1])
                a = do_agg(l, w, tab)
                prev = (w, a)
            if prev is not None:
                do_gru(l, prev[0], prev[1])

    nc.compile()
    return nc


# ---------------------------------------------------------------------------
# host wrappers
# ---------------------------------------------------------------------------
def _make_inputs(x, W, W_ih, W_hh, b_ih, b_hh, T, idx_rep, s_host, cnt):
    x = np.asarray(x, dtype=np.float32)
    W = np.asarray(W, dtype=np.float32)
    W_ih = np.asarray(W_ih, dtype=np.float32)
    W_hh = np.asarray(W_hh, dtype=np.float32)
    b_ih = np.asarray(b_ih, dtype=np.float32)
    b_hh = np.asarray(b_hh, dtype=np.float32)

    wie = np.zeros((128, L * 3 * 128), dtype=np.float32)
    for l in range(L):
        wi = W_ih @ W[l].T
        for k in range(3):
            wie[:, (l * 3 + k) * 128:(l * 3 + k + 1) * 128] = wi[k * 128:(k + 1) * 128, :].T
    whh = np.zeros((128, 3 * 128), dtype=np.float32)
    for k in range(3):
        whh[:, k * 128:(k + 1) * 128] = W_hh[k * 128:(k + 1) * 128, :].T
    bias = np.zeros((128, 4), dtype=np.float32)
    bias[:, 0] = b_ih[0:128] + b_hh[0:128]
    bias[:, 1] = b_ih[128:256] + b_hh[128:256]
    bias[:, 2] = b_ih[256:384]
    bias[:, 3] = b_hh[256:384]
    ident = np.eye(128, dtype=np.float32)

    # x table in chunked-allgather (pos) layout, split fp16 hi|lo
    hi, lo = _split16(x)
    xs = np.empty((N, 256), dtype=np.float16)
    posn = _pos_of(np.arange(N, dtype=np.int64))
    xs[posn, 0:128] = hi
    xs[posn, 128:256] = lo

    in_maps = []
    for c in range(C):
        x_ownT = np.zeros((D, NCP), dtype=np.float32)
        x_ownT[:, :NC_] = x[c * NC_:(c + 1) * NC_].T
        in_maps.append({
            "x_ownT": x_ownT,
            "x_split": xs,
            "s_dram": np.ascontiguousarray(s_host[c].reshape(128, -1)),
            "idx_dram": np.ascontiguousarray(idx_rep[c]),
            "wie_dram": wie,
            "whh_dram": whh,
            "bias_dram": bias,
            "ident_dram": ident,
            "cnt_dram": np.ascontiguousarray(cnt[c]),
        })
    return in_maps


_cache = {}


def kernel(x, edge_index, edge_attr, W, W_ih, W_hh, b_ih, b_hh):
    from concourse import bass_utils

    T, idx_rep, s_host, cnt = _plan(edge_index, edge_attr)
    if T not in _cache:
        _cache[T] = _build_program(T)
    nc = _cache[T]

    in_maps = _make_inputs(x, W, W_ih, W_hh, b_ih, b_hh, T, idx_rep, s_host, cnt)
    res = bass_utils.run_bass_kernel_spmd(nc, in_maps, list(range(C)))
    out = np.concatenate([res.results[c]["out"] for c in range(C)], axis=0)
    return out.astype(np.float32)
